# revision 13
# baseline (speedup 1.0000x reference)
"""Trainium2 Bass kernel for DenseDilatedKnnGraph (B=4, C=128, N=8192, k=9, dilation=4).

Strategy (v4: fp32r matmul + bf16 fold tree, ship all cells)
------------------------------------------------------------
reference: normalize x,y over channels; dist = |xn|^2 - 2<xn,yn> + |yn|^2 per
batch; edge_index[0] = top-36 by -dist (stable ties -> lower index) sampled
every 4th rank; edge_index[1] = arange(N).  |xn|^2 is constant per row and
|yn|^2 == 1 +- 1e-7, so ranking is by s = <xn, yn>.

Device (per core = one batch-half: 4096 query rows x 8192 candidates):
  - PE: fp32r matmuls (s accurate to ~7e-5; selection-grade — the final
    ranking is re-derived exactly on the host).
  - ACT (+ DVE for some chunks): PSUM->SBUF escape, cast to bf16.
  - DVE: three batched pairwise-max folds 8192 -> 1024 "cells" per row
    (bf16 tensor_tensor runs in 2x mode).  Cell j = max of the 8 scores at
    candidate positions 2048*(j>>8) + (j&255) + 256*k, k=0..7.
  - DMA ships all 1024 bf16 cells per row (8 MB/core, hidden under compute).

Host: top-48 cells per row by shipped value (cell id = column position),
expand each cell to its 8 member candidates, rescore those 384 exactly in
fp64, stable-sort for the top-36.  Correctness: a candidate with true rank
r has cell rank <= r (its cell's value >= its own), so top-48 covers the
top-36 with a >=12-cell margin against the ~7e-4 fp32r+bf16 noise
(~1 expected rank perturbation; P(miss) < 1e-10 per row).
"""

import os
import numpy as np

import concourse.bacc as bacc
import concourse.mybir as mybir
from concourse.tile import TileContext
from concourse.bass_utils import run_bass_kernel_spmd

# problem constants (hardcoded per harness contract)
B, C, N = 4, 128, 8192
K_OUT, DIL = 9, 4
KK = K_OUT * DIL            # 36
NQ = N // 2                 # 4096 query rows per core
TILES = NQ // 128           # 32
CH = 512                    # matmul free-dim chunk (one PSUM bank)
ECH = 2048                  # escape chunk (4 PSUM banks)
NECH = N // ECH             # 4 escape chunks per tile
CPC = 1024                  # cells per escape chunk (fold 2:1)
CELLS = NECH * CPC          # 1024 cells per row
NCAND_CELL = 48             # cells the host expands per row
EPS = 1e-12
F32 = mybir.dt.float32
F32R = mybir.dt.float32r
BF16 = mybir.dt.bfloat16
MAX = mybir.AluOpType.max

_CACHED = {}


def _build():
    nc = bacc.Bacc("TRN2")
    xs = nc.dram_tensor("xs", [C, NQ], F32R, kind="ExternalInput")
    yf = nc.dram_tensor("yf", [C, N], F32R, kind="ExternalInput")
    o_c = nc.dram_tensor("o_c", [TILES, 128, CELLS], BF16, kind="ExternalOutput")

    with TileContext(nc) as tc:
        with (
            tc.tile_pool(name="persist", bufs=1) as persist,
            tc.tile_pool(name="spool", bufs=4) as spool,
            tc.tile_pool(name="fpool", bufs=3) as fpool,
            tc.tile_pool(name="cpool", bufs=3) as cpool,
            tc.tile_pool(name="mpsum", bufs=2, space="PSUM") as mpsum,
        ):
            yn = persist.tile([C, N], F32R, tag="yn")
            xn = persist.tile([C, NQ], F32R, tag="xn")
            # chunked loads so tile 0's matmuls start after the first chunks
            nc.sync.dma_start(xn[:, :CH], xs[:, :CH])
            for j in range(N // CH):
                sl = slice(j * CH, (j + 1) * CH)
                nc.sync.dma_start(yn[:, sl], yf[:, sl])
            for j in range(1, NQ // CH):
                sl = slice(j * CH, (j + 1) * CH)
                nc.sync.dma_start(xn[:, sl], xs[:, sl])

            def emit_folds(t, S):
                # single fold, batched bf16 TT (2x mode): 8192 -> 4096 cells
                cells = cpool.tile([128, NECH, CPC], BF16, tag="cells")
                nc.vector.tensor_tensor(
                    cells[:, :, :], S[:, :, 0:ECH // 2], S[:, :, ECH // 2:ECH],
                    op=MAX)
                nc.sync.dma_start(o_c[t, :, :], cells[:, :, :])

            pending = None                       # (t, S) awaiting fold
            for t in range(TILES):
                lhsT = xn[:, t * 128:(t + 1) * 128]
                S = spool.tile([128, NECH, ECH], BF16, tag="S")
                for e in range(NECH):
                    ps = mpsum.tile([128, ECH], F32, tag="ps")
                    for k in range(ECH // CH):
                        psl = slice(k * CH, (k + 1) * CH)
                        ysl = slice(e * ECH + k * CH, e * ECH + (k + 1) * CH)
                        nc.tensor.matmul(ps[:, psl], lhsT, yn[:, ysl],
                                         start=True, stop=True)
                    # escape: PSUM -> SBUF bf16; DVE takes ~1.33 chunks per
                    # tile to balance ACT vs DVE load
                    if e == NECH - 1 or (e == NECH - 2 and t % 3 == 0):
                        nc.vector.tensor_copy(S[:, e, :], ps[:, :])
                    else:
                        nc.scalar.copy(S[:, e, :], ps[:, :])
                # software pipelining: fold the PREVIOUS tile now, so this
                # tile's PSUM-freeing escapes run ahead of bulk DVE work
                if pending is not None:
                    emit_folds(*pending)
                pending = (t, S)
            emit_folds(*pending)
    nc.finalize()
    return nc


def _host_normalize(t):
    # mimics reference._l2_normalize over axis 0 of a [C, N] f32 array
    n = np.sqrt(np.sum(t * t, axis=0, keepdims=True, dtype=np.float32),
                dtype=np.float32)
    return (t / np.maximum(n, np.float32(EPS))).astype(np.float32)


def kernel(x, y):
    x = np.ascontiguousarray(np.asarray(x, dtype=np.float32)[..., 0])  # (B, C, N)
    y = np.ascontiguousarray(np.asarray(y, dtype=np.float32)[..., 0])

    xn = np.stack([_host_normalize(x[b]) for b in range(B)])
    yn = np.stack([_host_normalize(y[b]) for b in range(B)])

    if "nc" not in _CACHED:
        _CACHED["nc"] = _build()
    nc = _CACHED["nc"]

    in_maps = []
    for k in range(8):
        b, h = k // 2, k % 2
        in_maps.append({
            "xs": np.ascontiguousarray(xn[b, :, h * NQ:(h + 1) * NQ]),
            "yf": yn[b],
        })

    trace = bool(int(os.environ.get("KNN_TRACE", "0")))
    res = run_bass_kernel_spmd(nc, in_maps, core_ids=list(range(8)), trace=trace)
    if res.exec_time_ns is not None:
        print(f"HW exec time: {res.exec_time_ns} ns")
        _CACHED["exec_time_ns"] = res.exec_time_ns

    # host: top-48 cells -> expand x8 -> exact fp64 rescore -> stable top-36
    nn_idx = np.zeros((B, N, KK), np.int32)
    koff = np.arange(2, dtype=np.int64) * CPC                 # within-chunk offsets
    for k in range(8):
        b, h = k // 2, k % 2
        out = res.results[k]
        cv = np.asarray(out["o_c"]).astype(np.float32).reshape(NQ, CELLS)
        sel = np.argpartition(-cv, NCAND_CELL, axis=1)[:, :NCAND_CELL]
        csel = sel.astype(np.int64)                           # cell id = position
        # expand: orig = 2048*(cell>>10) + (cell&1023) + 1024*k
        base = (csel >> 10) * ECH + (csel & (CPC - 1))        # [NQ, 48]
        cand = (base[:, :, None] + koff[None, None, :]).reshape(NQ, -1)  # [NQ,384]

        # exact fp64 rescore, chunked batched matmul (row-major gathers)
        xq = xn[b][:, h * NQ:(h + 1) * NQ].astype(np.float64)  # [C, NQ]
        ynbT = np.ascontiguousarray(yn[b].T.astype(np.float64))  # [N, C]
        top36 = np.empty((NQ, KK), np.int64)
        RCH = 512
        for r0 in range(0, NQ, RCH):
            r1 = min(r0 + RCH, NQ)
            idx = cand[r0:r1]                                  # [R, NC]
            Yg = ynbT[idx]                                     # [R, NC, C]
            A = xq[:, r0:r1].T[:, :, None]                     # [R, C, 1]
            s = np.matmul(Yg, A)[:, :, 0]                      # [R, NC]
            order = np.lexsort((idx, -s), axis=1)[:, :KK]
            top36[r0:r1] = np.take_along_axis(idx, order, axis=1)
        nn_idx[b, h * NQ:(h + 1) * NQ, :] = top36

    center = np.broadcast_to(np.arange(N, dtype=np.int32)[None, :, None],
                             (B, N, K_OUT))
    edge = np.stack([np.ascontiguousarray(nn_idx[:, :, ::DIL]), center], axis=0)
    return edge.astype(np.int32)


# revision 14
# speedup vs baseline: 1.2299x; 1.2299x over previous
"""Trainium2 Bass kernel for DenseDilatedKnnGraph (B=4, C=128, N=8192, k=9, dilation=4).

Strategy (v4: fp32r matmul + bf16 fold tree, ship all cells)
------------------------------------------------------------
reference: normalize x,y over channels; dist = |xn|^2 - 2<xn,yn> + |yn|^2 per
batch; edge_index[0] = top-36 by -dist (stable ties -> lower index) sampled
every 4th rank; edge_index[1] = arange(N).  |xn|^2 is constant per row and
|yn|^2 == 1 +- 1e-7, so ranking is by s = <xn, yn>.

Device (per core = one batch-half: 4096 query rows x 8192 candidates):
  - PE: fp32r matmuls (s accurate to ~7e-5; selection-grade — the final
    ranking is re-derived exactly on the host).
  - ACT (+ DVE for some chunks): PSUM->SBUF escape, cast to bf16.
  - DVE: three batched pairwise-max folds 8192 -> 1024 "cells" per row
    (bf16 tensor_tensor runs in 2x mode).  Cell j = max of the 8 scores at
    candidate positions 2048*(j>>8) + (j&255) + 256*k, k=0..7.
  - DMA ships all 1024 bf16 cells per row (8 MB/core, hidden under compute).

Host: top-48 cells per row by shipped value (cell id = column position),
expand each cell to its 8 member candidates, rescore those 384 exactly in
fp64, stable-sort for the top-36.  Correctness: a candidate with true rank
r has cell rank <= r (its cell's value >= its own), so top-48 covers the
top-36 with a >=12-cell margin against the ~7e-4 fp32r+bf16 noise
(~1 expected rank perturbation; P(miss) < 1e-10 per row).
"""

import os
import numpy as np

import concourse.bacc as bacc
import concourse.mybir as mybir
from concourse.tile import TileContext
from concourse.bass_utils import run_bass_kernel_spmd

# problem constants (hardcoded per harness contract)
B, C, N = 4, 128, 8192
K_OUT, DIL = 9, 4
KK = K_OUT * DIL            # 36
NQ = N // 2                 # 4096 query rows per core
TILES = NQ // 128           # 32
CH = 512                    # matmul free-dim chunk (one PSUM bank)
ECH = 2048                  # escape chunk (4 PSUM banks)
NECH = N // ECH             # 4 escape chunks per tile
CPC = 1024                  # cells per escape chunk (fold 2:1)
CELLS = NECH * CPC          # 1024 cells per row
NCAND_CELL = 48             # cells the host expands per row
EPS = 1e-12
F32 = mybir.dt.float32
F32R = mybir.dt.float32r
BF16 = mybir.dt.bfloat16
MAX = mybir.AluOpType.max

_CACHED = {}


def _build():
    nc = bacc.Bacc("TRN2")
    xs = nc.dram_tensor("xs", [C, NQ], F32R, kind="ExternalInput")
    yf = nc.dram_tensor("yf", [C, N], F32R, kind="ExternalInput")
    o_c = nc.dram_tensor("o_c", [TILES, 128, CELLS], BF16, kind="ExternalOutput")

    with TileContext(nc) as tc:
        with (
            tc.tile_pool(name="persist", bufs=1) as persist,
            tc.tile_pool(name="spool", bufs=4) as spool,
            tc.tile_pool(name="fpool", bufs=3) as fpool,
            tc.tile_pool(name="cpool", bufs=3) as cpool,
            tc.tile_pool(name="mpsum", bufs=4, space="PSUM") as mpsum,
        ):
            yn = persist.tile([C, N], F32R, tag="yn")
            xn = persist.tile([C, NQ], F32R, tag="xn")
            # chunked loads so tile 0's matmuls start after the first chunks
            nc.sync.dma_start(xn[:, :CH], xs[:, :CH])
            for j in range(N // CH):
                sl = slice(j * CH, (j + 1) * CH)
                nc.sync.dma_start(yn[:, sl], yf[:, sl])
            for j in range(1, NQ // CH):
                sl = slice(j * CH, (j + 1) * CH)
                nc.sync.dma_start(xn[:, sl], xs[:, sl])

            def emit_folds(t, S):
                # single fold, batched bf16 TT (2x mode): 8192 -> 4096 cells
                cells = cpool.tile([128, NECH, CPC], BF16, tag="cells")
                nc.vector.tensor_tensor(
                    cells[:, :, :], S[:, :, 0:ECH // 2], S[:, :, ECH // 2:ECH],
                    op=MAX)
                nc.sync.dma_start(o_c[t, :, :], cells[:, :, :])

            HCH = ECH // 2                       # 1024-col psum granule
            pending = None                       # (t, S) awaiting fold
            for t in range(TILES):
                lhsT = xn[:, t * 128:(t + 1) * 128]
                S = spool.tile([128, NECH, ECH], BF16, tag="S")
                for e in range(2 * NECH):
                    ps = mpsum.tile([128, HCH], F32, tag="ps")
                    for k in range(HCH // CH):
                        psl = slice(k * CH, (k + 1) * CH)
                        ysl = slice(e * HCH + k * CH, e * HCH + (k + 1) * CH)
                        nc.tensor.matmul(ps[:, psl], lhsT, yn[:, ysl],
                                         start=True, stop=True)
                    # escape: PSUM -> SBUF bf16; DVE takes ~2.3 of 8 granules
                    # (interleaved) to balance ACT vs DVE load
                    ssl = S[:, e // 2, (e % 2) * HCH:(e % 2 + 1) * HCH]
                    if e in (3, 7) or (e == 1 and t % 3 == 0):
                        nc.vector.tensor_copy(ssl, ps[:, :])
                    else:
                        nc.scalar.copy(ssl, ps[:, :])
                # software pipelining: fold the PREVIOUS tile now, so this
                # tile's PSUM-freeing escapes run ahead of bulk DVE work
                if pending is not None:
                    emit_folds(*pending)
                pending = (t, S)
            emit_folds(*pending)
    nc.finalize()
    return nc


def _host_normalize(t):
    # mimics reference._l2_normalize over axis 0 of a [C, N] f32 array
    n = np.sqrt(np.sum(t * t, axis=0, keepdims=True, dtype=np.float32),
                dtype=np.float32)
    return (t / np.maximum(n, np.float32(EPS))).astype(np.float32)


def kernel(x, y):
    x = np.ascontiguousarray(np.asarray(x, dtype=np.float32)[..., 0])  # (B, C, N)
    y = np.ascontiguousarray(np.asarray(y, dtype=np.float32)[..., 0])

    xn = np.stack([_host_normalize(x[b]) for b in range(B)])
    yn = np.stack([_host_normalize(y[b]) for b in range(B)])

    if "nc" not in _CACHED:
        _CACHED["nc"] = _build()
    nc = _CACHED["nc"]

    in_maps = []
    for k in range(8):
        b, h = k // 2, k % 2
        in_maps.append({
            "xs": np.ascontiguousarray(xn[b, :, h * NQ:(h + 1) * NQ]),
            "yf": yn[b],
        })

    trace = bool(int(os.environ.get("KNN_TRACE", "0")))
    res = run_bass_kernel_spmd(nc, in_maps, core_ids=list(range(8)), trace=trace)
    if res.exec_time_ns is not None:
        print(f"HW exec time: {res.exec_time_ns} ns")
        _CACHED["exec_time_ns"] = res.exec_time_ns

    # host: top-48 cells -> expand x8 -> exact fp64 rescore -> stable top-36
    nn_idx = np.zeros((B, N, KK), np.int32)
    koff = np.arange(2, dtype=np.int64) * CPC                 # within-chunk offsets
    for k in range(8):
        b, h = k // 2, k % 2
        out = res.results[k]
        cv = np.asarray(out["o_c"]).astype(np.float32).reshape(NQ, CELLS)
        sel = np.argpartition(-cv, NCAND_CELL, axis=1)[:, :NCAND_CELL]
        csel = sel.astype(np.int64)                           # cell id = position
        # expand: orig = 2048*(cell>>10) + (cell&1023) + 1024*k
        base = (csel >> 10) * ECH + (csel & (CPC - 1))        # [NQ, 48]
        cand = (base[:, :, None] + koff[None, None, :]).reshape(NQ, -1)  # [NQ,384]

        # exact fp64 rescore, chunked batched matmul (row-major gathers)
        xq = xn[b][:, h * NQ:(h + 1) * NQ].astype(np.float64)  # [C, NQ]
        ynbT = np.ascontiguousarray(yn[b].T.astype(np.float64))  # [N, C]
        top36 = np.empty((NQ, KK), np.int64)
        RCH = 512
        for r0 in range(0, NQ, RCH):
            r1 = min(r0 + RCH, NQ)
            idx = cand[r0:r1]                                  # [R, NC]
            Yg = ynbT[idx]                                     # [R, NC, C]
            A = xq[:, r0:r1].T[:, :, None]                     # [R, C, 1]
            s = np.matmul(Yg, A)[:, :, 0]                      # [R, NC]
            order = np.lexsort((idx, -s), axis=1)[:, :KK]
            top36[r0:r1] = np.take_along_axis(idx, order, axis=1)
        nn_idx[b, h * NQ:(h + 1) * NQ, :] = top36

    center = np.broadcast_to(np.arange(N, dtype=np.int32)[None, :, None],
                             (B, N, K_OUT))
    edge = np.stack([np.ascontiguousarray(nn_idx[:, :, ::DIL]), center], axis=0)
    return edge.astype(np.int32)
